# revision 15
# baseline (speedup 1.0000x reference)
"""Causal multi-head self-attention with RoPE on 8 Trainium2 NeuronCores.

Problem: x[2, 2048, 1024] fp32, 16 heads, d_head=64, causal, RoPE(theta=1e4).
Sharding: core = b*4 + g  (b in {0,1} batch, g in {0..3} head-group of 4 heads).
Each core computes out_partial[2048, 1024] = attn(heads of g) @ wo[:, cols_g].T;
host sums the 4 partials per batch.

Per-core kernel (matmul path in bf16, fp32 PSUM accumulation):
  B) Q/K projections into [d_head, seq] layout (2 heads per 128 partitions)
     with RoPE fused:  q_rot = A*cosT + P@(A*sinT)  (P = pair-swap sign matrix,
     applied via a single PE matmul; tables are pair-symmetric so P commutes
     with the elementwise sin multiply).
  C) V projection into [seq_tile(128) partitions, 4*64+ones] layout; the
     ones column makes the second attention matmul also produce the softmax
     denominator for free.
  D) Per (head-pair, q-chunk of 512): scores_T[k 128, q 512] = K_tile @ Q_chunk
     on PE (contraction d=64; the two heads of a pair use partition halves
     0:64/64:128 so their matmuls land in different PE row groups and run
     concurrently), exp on ACT over kt-PAIRS [128, 1024] (scale=1/8 fused),
     causal masking by 0/1-mask multiply on diagonal tiles, then
     attn_aug[65, 512] += V_aug.T @ probs_T accumulated in PSUM over k tiles.
     Normalize with reciprocal_approx_fast + DRAM-bounce partition broadcast.
  E) out_partial = attnT.T @ wo_t, tiled 128x512, accumulated over 2 k-subtiles.
"""

import os
import sys

sys.path.insert(0, "/opt/trn_rl_repo")

import ml_dtypes
import numpy as np

import concourse.bacc as bacc
import concourse.mybir as mybir
from concourse.tile import TileContext

B = 2
S = 2048
DM = 1024
H = 16
DH = 64
HLOC = 4  # heads per core
SC = 512  # q chunk size
NKT = S // 128  # 16 k tiles
NQC = S // SC  # 4 q chunks
P = 128
KO = DM // P  # 8 contraction subtiles for projections
SCALE = 1.0 / 8.0  # 1/sqrt(DH)
THETA = 10000.0

F32 = mybir.dt.float32
BF16 = mybir.dt.bfloat16

_CACHE = {}
DEBUG = False


def _build_nc():
    nc = bacc.Bacc("TRN2", enable_partition_id=False)
    Exp = mybir.ActivationFunctionType.Exp

    xT = nc.dram_tensor("xT", [DM, S], BF16, kind="ExternalInput")
    wq_t = nc.dram_tensor("wq_t", [DM, 256], BF16, kind="ExternalInput")
    wk_t = nc.dram_tensor("wk_t", [DM, 256], BF16, kind="ExternalInput")
    wv_t = nc.dram_tensor("wv_t", [DM, 256], BF16, kind="ExternalInput")
    wo_t = nc.dram_tensor("wo_t", [256, DM], BF16, kind="ExternalInput")
    cosT = nc.dram_tensor("cosT", [P, S], BF16, kind="ExternalInput")
    sinT = nc.dram_tensor("sinT", [P, S], BF16, kind="ExternalInput")
    perm = nc.dram_tensor("perm", [P, P], BF16, kind="ExternalInput")
    tri = nc.dram_tensor("tri", [P, P], BF16, kind="ExternalInput")
    outp = nc.dram_tensor("out_partial", [S, DM], BF16, kind="ExternalOutput")

    with TileContext(nc) as tc:
        with tc.tile_pool(name="persist", bufs=1) as persist:
            # [pair-head-dim (2*64), head-pair, seq]
            q_rot = persist.tile([P, 2, S], BF16, tag="q_rot")
            k_rot = persist.tile([P, 2, S], BF16, tag="k_rot")
            # V in [k partitions, k_tile, head, 72]: cols 0:64 = V, 64 = ones
            v_sb = persist.tile([P, NKT, HLOC, 72], BF16, tag="v_sb")
            # attention output, transposed: [head-dim rows, ko, seq]
            attnT = persist.tile([P, 2, S], BF16, tag="attnT")
            wo_sb = persist.tile([P, 2, DM], BF16, tag="wo_sb")
            tri_sb = persist.tile([P, P], BF16, tag="tri_sb")

            # ---------------- Phase B/C: projections + rope + V -------------
            with tc.tile_pool(name="bc", bufs=1) as bc, \
                 tc.tile_pool(name="bcw", bufs=3) as bcw, \
                 tc.tile_pool(name="bcp", bufs=8, space="PSUM") as bcp:
                # weights + tables on the scalar engine's DMA queue, per-ko
                # chunks so wq[ko] lands just ahead of projection round ko
                wq_sb = bc.tile([P, KO, 256], BF16, tag="wq_sb")
                wk_sb = bc.tile([P, KO, 256], BF16, tag="wk_sb")
                wv_sb = bc.tile([P, KO, 256], BF16, tag="wv_sb")
                for t, d in ((wq_sb, wq_t), (wk_sb, wk_t), (wv_sb, wv_t)):
                    d_ap = d[:].rearrange("(ko p) m -> p ko m", p=P)
                    for ko in range(KO):
                        nc.scalar.dma_start(t[:, ko, :], d_ap[:, ko, :])
                cos_sb = bc.tile([P, S], BF16, tag="cos_sb")
                sin_sb = bc.tile([P, S], BF16, tag="sin_sb")
                nc.scalar.dma_start(cos_sb[:], cosT[:])
                nc.scalar.dma_start(sin_sb[:], sinT[:])
                perm_sb = bc.tile([P, P], BF16, tag="perm_sb")
                nc.scalar.dma_start(perm_sb[:], perm[:])

                # x chunks sequentially on the sync queue: chunk ko arrives
                # ~1.6us apart, pacing the ko-outer Q projection rounds
                xT_sb = bc.tile([P, KO, S], BF16, tag="xT_sb")
                xT_ap = xT[:].rearrange("(ko p) s -> p ko s", p=P)
                for ko in range(KO):
                    nc.sync.dma_start(xT_sb[:, ko, :], xT_ap[:, ko, :])
                nc.sync.dma_start(tri_sb[:], tri[:])
                nc.sync.dma_start(
                    wo_sb[:], wo_t[:].rearrange("(ko p) m -> p ko m", p=P)
                )

                # ones column for the denominator trick
                nc.vector.memset(v_sb[:, :, :, 64:65], 1.0)

                def rope(a_ps, dest, cs_):
                    # dest = a*cos + P@(a*sin); a staged to bf16 once on ACT
                    a_sb = bcw.tile([P, SC], BF16, tag="a_sb")
                    nc.scalar.copy(out=a_sb[:], in_=a_ps[:])
                    t2 = bcw.tile([P, SC], BF16, tag="t2", bufs=2)
                    nc.vector.tensor_mul(
                        out=t2[:], in0=a_sb[:], in1=sin_sb[:, cs_]
                    )
                    b_ps = bcp.tile([P, SC], F32, tag="proj", name="b_ps")
                    nc.tensor.matmul(
                        b_ps[:], lhsT=perm_sb[:], rhs=t2[:],
                        start=True, stop=True,
                    )
                    nc.vector.tensor_mul(
                        out=dest, in0=a_sb[:], in1=cos_sb[:, cs_]
                    )
                    nc.vector.tensor_add(out=dest, in0=dest, in1=b_ps[:])

                # Q projection ko-outer across all 8 (hp, qc) tiles: the PE
                # follows the x-chunk DMA stream with no long head stall
                qa = [
                    bcp.tile([P, SC], F32, tag="proj", name=f"qa{i}")
                    for i in range(8)
                ]
                for ko in range(KO):
                    for hp in range(2):
                        for qc in range(NQC):
                            nc.tensor.matmul(
                                qa[hp * NQC + qc][:],
                                lhsT=wq_sb[:, ko, hp * P:(hp + 1) * P],
                                rhs=xT_sb[:, ko, qc * SC:(qc + 1) * SC],
                                start=(ko == 0),
                                stop=(ko == KO - 1),
                            )
                for hp in range(2):
                    for qc in range(NQC):
                        cs = slice(qc * SC, (qc + 1) * SC)
                        rope(qa[hp * NQC + qc], q_rot[:, hp, cs], cs)

                # K projection (x resident), ko-inner per tile
                for hp in range(2):
                    for qc in range(NQC):
                        cs = slice(qc * SC, (qc + 1) * SC)
                        ka = bcp.tile([P, SC], F32, tag="proj", name="ka")
                        for ko in range(KO):
                            nc.tensor.matmul(
                                ka[:],
                                lhsT=wk_sb[:, ko, hp * P:(hp + 1) * P],
                                rhs=xT_sb[:, ko, qc * SC:(qc + 1) * SC],
                                start=(ko == 0),
                                stop=(ko == KO - 1),
                            )
                        rope(ka, k_rot[:, hp, cs], cs)

                # V projection
                for st in range(NKT):
                    v_ps = bcp.tile([P, 256], F32, tag="proj", name="v_ps")
                    for ko in range(KO):
                        nc.tensor.matmul(
                            v_ps[:],
                            lhsT=xT_sb[:, ko, st * P:(st + 1) * P],
                            rhs=wv_sb[:, ko, :],
                            start=(ko == 0),
                            stop=(ko == KO - 1),
                        )
                    nc.vector.tensor_copy(
                        out=v_sb[:, st, :, 0:64],
                        in_=v_ps[:].rearrange("p (h d) -> p h d", d=DH),
                    )

            # ---------------- Phase D: attention (+ fused out proj) ---------
            with tc.tile_pool(name="dp", bufs=1) as dp, \
                 tc.tile_pool(name="dw", bufs=6) as dw, \
                 tc.tile_pool(name="dn", bufs=2) as dn, \
                 tc.tile_pool(name="dps", bufs=2, space="PSUM") as dps, \
                 tc.tile_pool(name="dpa", bufs=2, space="PSUM") as dpa, \
                 tc.tile_pool(name="ddr", bufs=4, space="DRAM") as ddr:
                out_ap = outp[:].rearrange("(st p) m -> p st m", p=P)

                def phase_e(qc_):
                    # output projection for chunk qc_ (both hp normalized);
                    # o_ps borrows the attn-tag rings
                    for st in range(4 * qc_, 4 * qc_ + 4):
                        ob = dn.tile([P, DM], BF16, tag="ob")
                        for no in range(2):
                            o_ps = dpa.tile([P, SC], F32, tag=f"attn{no}",
                                            name="o_ps")
                            for ko in range(2):
                                nc.tensor.matmul(
                                    o_ps[:],
                                    lhsT=attnT[:, ko, st * P:(st + 1) * P],
                                    rhs=wo_sb[:, ko, no * SC:(no + 1) * SC],
                                    start=(ko == 0),
                                    stop=(ko == 1),
                                )
                            nc.vector.tensor_copy(
                                out=ob[:, no * SC:(no + 1) * SC],
                                in_=o_ps[:],
                            )
                        nc.sync.dma_start(out_ap[:, st, :], ob[:])

                for hp in range(2):
                    for qc in range(NQC):
                        if hp == 1 and qc >= 1:
                            phase_e(qc - 1)
                        cs = slice(qc * SC, (qc + 1) * SC)
                        nkt_v = 4 * qc + 4
                        at_ps = [
                            dpa.tile([65, SC], F32, tag=f"attn{hh}",
                                     name=f"at_ps{hh}")
                            for hh in range(2)
                        ]
                        for kp in range(nkt_v // 2):
                            s2 = [
                                dps.tile([P, 2, SC], F32, tag="scores",
                                         name=f"s2_{hh2}")
                                for hh2 in range(2)
                            ]
                            for j in range(2):
                                kt = 2 * kp + j
                                r = kt - 4 * qc
                                w0 = 128 * r if r > 0 else 0
                                for hh in range(2):
                                    hs = slice(hh * 64, (hh + 1) * 64)
                                    nc.tensor.matmul(
                                        s2[hh][:, j, w0:SC],
                                        lhsT=k_rot[hs, hp,
                                                   kt * P:(kt + 1) * P],
                                        rhs=q_rot[hs, hp,
                                                  qc * SC + w0:(qc + 1) * SC],
                                        start=True,
                                        stop=True,
                                    )
                            r1 = 2 * kp + 1 - 4 * qc
                            for hh in range(2):
                                h = 2 * hp + hh
                                pt = dw.tile([P, 2, SC], BF16, tag="pt",
                                             name="pt")
                                if r1 < 0:
                                    # both k-tiles fully below the diagonal
                                    nc.scalar.activation(
                                        out=pt[:], in_=s2[hh][:], func=Exp,
                                        scale=SCALE,
                                    )
                                else:
                                    for j in range(2):
                                        r = 2 * kp + j - 4 * qc
                                        w0 = 128 * r if r > 0 else 0
                                        if w0 > 0:
                                            nc.gpsimd.memset(
                                                pt[:, j, 0:w0], 0.0
                                            )
                                        nc.scalar.activation(
                                            out=pt[:, j, w0:SC],
                                            in_=s2[hh][:, j, w0:SC],
                                            func=Exp, scale=SCALE,
                                        )
                                for j in range(2):
                                    r = 2 * kp + j - 4 * qc
                                    if r >= 0:
                                        w0 = 128 * r
                                        nc.vector.tensor_mul(
                                            out=pt[:, j, w0:w0 + 128],
                                            in0=pt[:, j, w0:w0 + 128],
                                            in1=tri_sb[:],
                                        )
                                for j in range(2):
                                    kt = 2 * kp + j
                                    nc.tensor.matmul(
                                        at_ps[hh][:],
                                        lhsT=v_sb[:, kt, h, 0:65],
                                        rhs=pt[:, j, :],
                                        start=(kt == 0),
                                        stop=(kt == nkt_v - 1),
                                    )
                        # normalize: rows 0:64 are attn, row 64 is denom;
                        # 1/denom partition-broadcast via DRAM bounce on the
                        # scalar DMA queue (sync queue carries outputs)
                        for hh in range(2):
                            rd = dn.tile([P, SC], F32, tag="rd")
                            nc.vector.tensor_copy(
                                out=rd[64:65, :], in_=at_ps[hh][64:65, :]
                            )
                            dr = ddr.tile([1, SC], F32, tag="dr")
                            nc.scalar.dma_start(dr[:], rd[64:65, :])
                            den_bc = dn.tile([64, SC], F32, tag="den_bc")
                            nc.scalar.dma_start(
                                den_bc[:], dr[:].partition_broadcast(64)
                            )
                            rbc = dn.tile([64, SC], F32, tag="rbc")
                            nc.vector.reciprocal_approx_fast(
                                out=rbc[:], in_=den_bc[:]
                            )
                            if hh == 0:
                                nc.vector.tensor_mul(
                                    out=attnT[0:64, hp, cs],
                                    in0=at_ps[hh][0:64, :],
                                    in1=rbc[:],
                                )
                            else:
                                tmp = dn.tile([64, SC], BF16, tag="tmp")
                                nc.vector.tensor_mul(
                                    out=tmp[:], in0=at_ps[hh][0:64, :],
                                    in1=rbc[:],
                                )
                                nc.scalar.dma_start(
                                    attnT[64:128, hp, cs], tmp[:]
                                )
                phase_e(3)
    nc.compile()
    return nc


def _host_tables(token_positions):
    pos = np.asarray(token_positions).astype(np.float64)
    freq = 1.0 / (THETA ** (2.0 * np.arange(DH // 2, dtype=np.float64) / DH))
    ang = pos[:, None] * freq[None, :]  # [S, 32]
    cos_f = np.repeat(np.cos(ang), 2, axis=1)  # [S, 64]
    sin_f = np.repeat(np.sin(ang), 2, axis=1)
    cosT = np.ascontiguousarray(
        np.concatenate([cos_f.T, cos_f.T], axis=0)
    ).astype(ml_dtypes.bfloat16)  # [128, S]
    sinT = np.ascontiguousarray(
        np.concatenate([sin_f.T, sin_f.T], axis=0)
    ).astype(ml_dtypes.bfloat16)

    perm = np.zeros((P, P), dtype=ml_dtypes.bfloat16)
    for i in range(P // 2):
        perm[2 * i + 1, 2 * i] = -1.0
        perm[2 * i, 2 * i + 1] = 1.0

    p_idx = np.arange(P)[:, None]
    f_idx = np.arange(P)[None, :]
    tri = (f_idx >= p_idx).astype(ml_dtypes.bfloat16)  # [128, 128]
    return cosT, sinT, perm, tri


_LAST_RESULTS = None


def _bf16(a):
    return np.ascontiguousarray(a).astype(ml_dtypes.bfloat16)


def kernel(x, wq, wk, wv, wo, token_positions):
    global _LAST_RESULTS
    from concourse.bass_utils import run_bass_kernel_spmd

    if "nc" not in _CACHE:
        _CACHE["nc"] = _build_nc()
    nc = _CACHE["nc"]

    x = np.asarray(x, dtype=np.float32)
    wq = np.asarray(wq, dtype=np.float32)
    wk = np.asarray(wk, dtype=np.float32)
    wv = np.asarray(wv, dtype=np.float32)
    wo = np.asarray(wo, dtype=np.float32)
    cosT, sinT, perm, tri = _host_tables(token_positions)

    in_maps = []
    for b in range(B):
        xT_b = _bf16(x[b].T)  # [DM, S]
        for g in range(4):
            rows = slice(g * 256, (g + 1) * 256)
            in_maps.append(
                {
                    "xT": xT_b,
                    "wq_t": _bf16(wq[rows].T),
                    "wk_t": _bf16(wk[rows].T),
                    "wv_t": _bf16(wv[rows].T),
                    "wo_t": _bf16(wo[:, rows].T),
                    "cosT": cosT,
                    "sinT": sinT,
                    "perm": perm,
                    "tri": tri,
                }
            )

    res = run_bass_kernel_spmd(
        nc,
        in_maps,
        core_ids=list(range(8)),
        trace=bool(os.environ.get("BASS_TRACE")),
    )
    _LAST_RESULTS = res
    outs = res.results

    out = np.zeros((B, S, DM), dtype=np.float32)
    for b in range(B):
        for g in range(4):
            out[b] += np.asarray(
                outs[b * 4 + g]["out_partial"], dtype=np.float32
            )
    return out

